# revision 1
# baseline (speedup 1.0000x reference)
"""BertSelfAttention on 8 Trainium2 NeuronCores.

Sharding: 8 cores = 4 batches x 2 head-halves. Each core computes, for its
batch b and its 8 heads, the unnormalized attention output transposed
(out.T = V.T @ P.T per head) plus the softmax denominator row (via a ones
column appended to V). The host pre-transposes inputs (X.T, W.T slices,
cast to fp16) and does the final normalize/transpose/concat.

Dtypes: fp16 operands for all matmuls (PE 1 cyc/row), fp32 PSUM
accumulation, exp on ScalarE from PSUM (scale=1/8 fused), fp32 output.

Schedule: phase 1 (projections) is PE-bound (~100us incl ramp); phase 2
(attention) runs at the ScalarE exp floor (~260us, ScalarE 100% busy):
the PE stream is software-pipelined one k-step behind (scores k+1 ahead
of AV k), and parity-swapped duplicates of Q.T/K.T (built with SBUF->SBUF
partition-shift DMAs) let consecutive score matmuls alternate PE row
groups so LDWEIGHTS hides under in-flight matmuls. ~370us total on HW.
"""

import sys

if "/opt/trn_rl_repo" not in sys.path:
    sys.path.insert(0, "/opt/trn_rl_repo")

import numpy as np

import concourse.bass as bass  # noqa: F401  (registers bass machinery)
import concourse.tile as tile
from concourse import bacc, mybir
from concourse.bass_utils import run_bass_kernel_spmd

B, S, H = 4, 2048, 1024
NH, DH = 16, 64
NCORES = 8
HPC = 8            # heads per core
OC = HPC * DH      # 512 output features per core
HC = H // 128      # 8 contraction chunks of 128
DHE = DH + 1       # head dim + denominator column

F16 = mybir.dt.float16
F32 = mybir.dt.float32
F32R = mybir.dt.float32r
EXP = mybir.ActivationFunctionType.Exp

_PROGRAM = None
LAST_RESULT = None  # BassKernelResults of the most recent kernel() call


def _emit_kernel(tc, out, xt, wqt, wkt, wvt):
    nc = tc.nc
    with (
        tc.tile_pool(name="persist", bufs=1) as persist,
        tc.tile_pool(name="ptp", bufs=5) as ptp,
        tc.tile_pool(name="ost", bufs=4) as ost,
        # one PSUM pool for both phases: projections borrow the score slots
        # (ps0/ps1) so there is no pool-transition serialization.
        tc.tile_pool(name="psa", bufs=1, space="PSUM") as psa,
    ):
        xt_sb = persist.tile([128, HC, S], F16)
        wq_sb = persist.tile([128, HC, OC], F16)
        wk_sb = persist.tile([128, HC, OC], F16)
        wv_sb = persist.tile([128, HC, OC], F16)
        qt_sb = persist.tile([128, 4, S], F16)
        kt_sb = persist.tile([128, 4, S], F16)
        # parity-swapped duplicates: head at rows 0-63 in qt_sb sits at rows
        # 64-127 here (and vice versa), so the two q-halves of one head's
        # score matmuls target different PE row groups -> LDWEIGHTS hides
        # under the in-flight matmul and the matmuls themselves overlap.
        qt2_sb = persist.tile([128, 4, S], F16)
        kt2_sb = persist.tile([128, 4, S], F16)
        v_sb = persist.tile([128, 16, HPC * DHE], F16)

        xt_chunks = xt.rearrange("(c p) s -> p c s", p=128)
        wv_chunks = wvt.rearrange("(c p) o -> p c o", p=128)
        for hc in range(HC):
            nc.sync.dma_start(wv_sb[:, hc, :], wv_chunks[:, hc, :])
            nc.sync.dma_start(xt_sb[:, hc, :], xt_chunks[:, hc, :])
        nc.sync.dma_start(wq_sb[:], wqt.rearrange("(c p) o -> p c o", p=128))
        nc.sync.dma_start(wk_sb[:], wkt.rearrange("(c p) o -> p c o", p=128))

        # fill V with ones first; projection copies overwrite the data columns,
        # leaving a ones column per head to accumulate softmax denominators
        nc.vector.memset(v_sb[:], 1.0)

        P1TAGS = ("ps0", "ps1", "po0", "po1")

        def proj_tile(idx, w_sb, c, sc, dst, tag=None, engine=None):
            tag = tag if tag is not None else P1TAGS[idx % 4]
            p = psa.tile([128, 1024], F32, tag=tag, name=f"pp_{tag}")
            for hc in range(HC):
                nc.tensor.matmul(
                    p[:, 0:512],
                    w_sb[:, hc, c * 128 : (c + 1) * 128],
                    xt_sb[:, hc, sc * 512 : (sc + 1) * 512],
                    start=(hc == 0),
                    stop=(hc == HC - 1),
                )
            nc.vector.tensor_copy(dst[:, c, sc * 512 : (sc + 1) * 512], p[:, 0:512])

        def v_tile(idx, st):
            p = psa.tile([128, 1024], F32, tag=P1TAGS[idx % 4], name=f"pv{idx % 4}")
            for hc in range(HC):
                nc.tensor.matmul(
                    p[:, 0:512],
                    xt_sb[:, hc, st * 128 : (st + 1) * 128],
                    wv_sb[:, hc, :],
                    start=(hc == 0),
                    stop=(hc == HC - 1),
                )
            nc.vector.tensor_copy(
                v_sb[:, st, :].rearrange("p (h e) -> p h e", e=DHE)[:, :, 0:DH],
                p[:, 0:512].rearrange("p (h d) -> p h d", d=DH),
            )

        def swap_dmas(c, lo, hi):
            # parity-swapped duplicates via SBUF->SBUF partition-shift DMAs
            # (engines cannot move data across partitions; DMA can)
            for src, dst in ((qt_sb, qt2_sb), (kt_sb, kt2_sb)):
                nc.sync.dma_start(dst[0:64, c, lo:hi], src[64:128, c, lo:hi])
                nc.sync.dma_start(dst[64:128, c, lo:hi], src[0:64, c, lo:hi])

        # ---- phase 1: projections (V first, then Q/K) ----
        n = 0
        for st in range(16):
            v_tile(n, st)
            n += 1
        for c in range(4):
            for w_sb, dst in ((wq_sb, qt_sb), (wk_sb, kt_sb)):
                for sc in range(4):
                    proj_tile(n, w_sb, c, sc, dst)
                    n += 1
            swap_dmas(c, 0, S)

        # ---- phase 2: attention (head pairs packed in PE row groups) ----
        for pair in range(HPC // 2):
            chunk = pair
            for qb in range(2):         # q blocks of 1024
                po = [psa.tile([DHE, 1024], F32, tag=f"po{p}", name=f"po{p}") for p in range(2)]

                def av(k, pts):
                    for p in range(2):
                        hsl = slice((2 * pair + p) * DHE, (2 * pair + p + 1) * DHE)
                        for q2 in range(2):
                            nc.tensor.matmul(
                                po[p][:, q2 * 512 : (q2 + 1) * 512],
                                v_sb[:, k, hsl],
                                pts[p][:, q2 * 512 : (q2 + 1) * 512],
                                start=(k == 0),
                                stop=(k == 15),
                            )

                pending = []  # (k, pt-pair): AV lags scores by one k step
                for k in range(16):     # key tiles of 128
                    ksl = slice(k * 128, (k + 1) * 128)
                    ps = [psa.tile([128, 1024], F32, tag=f"ps{p}", name=f"ps{p}") for p in range(2)]
                    pt = [ptp.tile([128, 1024], F16, tag=f"pt{p}", name=f"pt{p}") for p in range(2)]
                    # q2=0 reads the primary layout, q2=1 the parity-swapped
                    # duplicate: every consecutive score matmul (within a head
                    # and across heads) alternates PE row groups
                    for q2 in range(2):
                        q0 = qb * 1024 + q2 * 512
                        for p in range(2):  # head parity: rows 0-63 / 64-127
                            base = (p if q2 == 0 else 1 - p) * 64
                            kt_src = kt_sb if q2 == 0 else kt2_sb
                            qt_src = qt_sb if q2 == 0 else qt2_sb
                            nc.tensor.matmul(
                                ps[p][:, q2 * 512 : (q2 + 1) * 512],
                                kt_src[base : base + 64, chunk, ksl],
                                qt_src[base : base + 64, chunk, q0 : q0 + 512],
                                start=True,
                                stop=True,
                            )
                    for p in range(2):
                        nc.scalar.activation(pt[p][:], ps[p][:], EXP, scale=0.125)
                    pending.append((k, pt))
                    if len(pending) > 1:
                        av(*pending.pop(0))
                for item in pending:
                    av(*item)
                for p in range(2):
                    o = ost.tile([DHE, 1024], F32, tag="o")
                    for h2 in range(2):
                        hs = slice(h2 * 512, (h2 + 1) * 512)
                        nc.vector.tensor_copy(o[:, hs], po[p][:, hs])
                        nc.sync.dma_start(
                            out[2 * pair + p, :, qb * 1024 + h2 * 512 : qb * 1024 + (h2 + 1) * 512],
                            o[:, hs],
                        )


def _get_program():
    global _PROGRAM
    if _PROGRAM is None:
        nc = bacc.Bacc(
            "TRN2", target_bir_lowering=False, debug=False, num_devices=NCORES
        )
        xt = nc.dram_tensor("xt", [H, S], F16, kind="ExternalInput").ap()
        wqt = nc.dram_tensor("wqt", [H, OC], F16, kind="ExternalInput").ap()
        wkt = nc.dram_tensor("wkt", [H, OC], F16, kind="ExternalInput").ap()
        wvt = nc.dram_tensor("wvt", [H, OC], F16, kind="ExternalInput").ap()
        out = nc.dram_tensor("out", [HPC, DHE, S], F32, kind="ExternalOutput").ap()
        with tile.TileContext(nc) as tc:
            _emit_kernel(tc, out, xt, wqt, wkt, wvt)
        nc.compile()
        _PROGRAM = nc
    return _PROGRAM


def kernel(**inputs):
    global LAST_RESULT
    X = np.asarray(inputs["hidden_states"], dtype=np.float32)
    Ws = {k: np.asarray(inputs[k], dtype=np.float32) for k in ("Wq", "Wk", "Wv")}

    nc = _get_program()
    in_maps = []
    for core in range(NCORES):
        b, half = core // 2, core % 2
        sl = slice(half * OC, (half + 1) * OC)
        in_maps.append(
            {
                "xt": np.ascontiguousarray(X[b].T).astype(np.float16),
                "wqt": np.ascontiguousarray(Ws["Wq"][sl].T).astype(np.float16),
                "wkt": np.ascontiguousarray(Ws["Wk"][sl].T).astype(np.float16),
                "wvt": np.ascontiguousarray(Ws["Wv"][sl].T).astype(np.float16),
            }
        )

    LAST_RESULT = run_bass_kernel_spmd(nc, in_maps, core_ids=list(range(NCORES)))

    out = np.empty((B, S, H), dtype=np.float32)
    for core in range(NCORES):
        r = LAST_RESULT.results[core]["out"]          # [HPC, DHE, S]
        num = r[:, :DH, :]                            # [8, 64, 2048]
        den = r[:, DH : DH + 1, :]                    # [8, 1, 2048]
        o = (num / den).transpose(2, 0, 1).reshape(S, OC)
        b, half = core // 2, core % 2
        out[b, :, half * OC : (half + 1) * OC] = o
    return out

